# revision 34
# baseline (speedup 1.0000x reference)
"""AttentionPooling (segment_reduce) Trainium2 kernel.

att = sigmoid([input_rep, final_rep] @ W_lin.T + b_lin)
g   = att * (final_rep @ W_last.T + b_last)
out = segment_sum(g, graph_index, 16384)          # graph_index sorted

Strategy (8 NeuronCores, pure data-parallel, no collectives):
  graph_index is sorted, so a contiguous node range covers a contiguous
  graph range.  Host greedily packs whole graphs into "windows" of
  <= WIN_NODES nodes spanning <= 128 graphs; ~136 windows cover all 500k
  nodes = 8 cores x 17 windows.  Each core gets its windows as a padded
  node stream in feature-major bf16 layout.

Per 128-node subtile the device does (all psum MMs in one bank
[128, 512]: att columns 0:256, val columns 256:512):
    xf1.T @ [Wlin2|Wlast1]      (bf16, N=512, start)
    [xin;xf0].T @ [Wlin0;Wlin1] (fp8e4m3 DoubleRow, K=256, att cols)
    xf0.T @ Wlast0              (bf16, val cols, N=256)
    ones128.T @ bval/128        (val cols, N=256, K=128)  <- val bias
    ACT: att = sigmoid(psum att cols)     -> bf16
    DVE: g = att * psum val cols          -> bf16
    PE : oh.T @ g  += seg_psum[128 graphs, 256]   (lags 3 subtiles)
The attention pre-activation tolerates fp8: rel err 1.2e-2 vs the 2e-2
gate (measured on hardware == numpy simulation of e4m3 quantization).
The val path must stay bf16 (fp8 there costs ~2.5e-2).  The bias
matmul is K=128 (all-ones stationary, b'/128 replicated rows moving)
so every PE instruction keeps the same 128-row tile config — a K=1
matmul forces a row-config transition that stalls the next two
matmuls' weight loads (~2x107ns, measured).

b_lin is folded into the inputs on the host: solve W_lin @ s = b_lin
(min-norm; W_lin is a well-conditioned wide Gaussian) and shift every
node's features by s.  Then sigmoid(x'@W_lin.T) == sigmoid(x@W_lin.T +
b_lin) exactly, killing the att-half bias matmul.  The spill into the
val path is absorbed into the device-side val bias:
b' = b_last - s[128:] @ W_last.T.

The one-hot matrices are built on the host (exact in bf16) and DMA'd,
keeping DVE at the gating multiply only.  Seg matmuls lag the body by
SEGLAG subtiles and flow across window boundaries (seg psum is double
buffered), so the in-order PE queue never waits on the ACT->DVE chain.
"""

import numpy as np
import ml_dtypes

import concourse.bacc as bacc
import concourse.tile as tile
from concourse import mybir
from concourse import bass_utils
from concourse._compat import with_exitstack

P = 128
HID = 256
WIN_SUB = 29                     # subtiles (128 nodes) per window
WIN_NODES = WIN_SUB * P          # 3712
WINDOWS_PER_CORE = 17
N_CORES = 8
NUM_GRAPHS = 16384
GMAX = P                         # graph span per window
SEGLAG = 3                       # seg MM trails the body by this many subtiles

BF16 = mybir.dt.bfloat16
F32 = mybir.dt.float32
FP8 = mybir.dt.float8e4
npbf16 = ml_dtypes.bfloat16
npf8 = ml_dtypes.float8_e4m3

CHUNKS0 = [2, 6, WIN_SUB - 8]    # window-0 DMA chunking (subtiles)
CONST_W = 512 + 256 + 128 + 256  # [wcat1|wlast0|ones|bval]


# ----------------------------------------------------------------------------
# host-side planning
# ----------------------------------------------------------------------------

def _build_windows(gi: np.ndarray, num_graphs: int):
    """Greedy windows: contiguous whole-graph ranges, graph span <= GMAX,
    node count <= WIN_NODES.  Returns list of (gbase, gcnt, nstart, ncnt)."""
    counts = np.bincount(gi, minlength=num_graphs)
    starts = np.concatenate([[0], np.cumsum(counts)])
    wins = []
    g = 0
    while g < num_graphs:
        base = g
        nodes = 0
        cnt = 0
        while g < num_graphs and cnt < GMAX and nodes + counts[g] <= WIN_NODES:
            nodes += int(counts[g])
            cnt += 1
            g += 1
        if cnt == 0:
            raise ValueError(f"graph {g} has {counts[g]} nodes > {WIN_NODES}")
        wins.append((base, cnt, int(starts[base]), nodes))
    return wins


# ----------------------------------------------------------------------------
# device kernel
# ----------------------------------------------------------------------------

@with_exitstack
def _device_kernel(ctx, tc, out_ap, ins, n_windows):
    nc = tc.nc
    xall_ap, xpr_ap, wconst_ap, w8_ap = ins

    consts = ctx.enter_context(tc.tile_pool(name="consts", bufs=1))
    xpool = ctx.enter_context(tc.tile_pool(name="x", bufs=2))
    x0pool = ctx.enter_context(tc.tile_pool(name="x0", bufs=1))
    apool = ctx.enter_context(tc.tile_pool(name="act", bufs=6))
    gpool = ctx.enter_context(tc.tile_pool(name="g", bufs=6))
    outpool = ctx.enter_context(tc.tile_pool(name="out", bufs=2))
    ps_sub = ctx.enter_context(tc.tile_pool(name="ps_sub", bufs=6, space="PSUM"))
    ps_seg = ctx.enter_context(tc.tile_pool(name="ps_seg", bufs=2, space="PSUM"))

    # SDMA drains transfers in doorbell order across all queues, so the
    # first-needed data must doorbell first: window-0 chunk 0 goes out on
    # Sync before anything else; the constants ride the Scalar HWDGE
    # queue (doorbells land between Sync's chunk-0 and chunk-1 issues).
    wconst = consts.tile([P, CONST_W], BF16)
    wcat1 = wconst[:, 0:512]
    wlast0 = wconst[:, 512:768]
    ones_t = wconst[:, 768:896]
    bval = wconst[:, 896:1152]
    w8 = consts.tile([P, 2, HID], FP8)

    def load_consts():
        nc.scalar.dma_start(wconst[:], wconst_ap[:])
        nc.scalar.dma_start(w8[:], w8_ap[:])

    n_sub = n_windows * WIN_SUB

    # per-window input: one bf16 DMA [xf0|xf1|oh] + one fp8 DMA [xin|xf0]
    # (chunk-major for window 0 so the PE starts after ~0.3 MB)
    xpr_t = [None] * n_windows
    xf0_t = [None] * n_windows
    xf1_t = [None] * n_windows
    oh_t = [None] * n_windows

    def load_window(w):
        """x*_t[w][s] hold (tile, column offset of this subtile's block)."""
        if w == 0:
            tiles, ptiles, cb = [], [], []
            c0 = 0
            for q, csub in enumerate(CHUNKS0):
                cw = csub * P
                t = x0pool.tile([P, 3 * cw], BF16, tag=f"xc{q}")
                nc.sync.dma_start(t[:], xall_ap[:, 3 * c0 * P:3 * (c0 * P + cw)])
                pt = x0pool.tile([P, 2, cw], FP8, tag=f"xp{q}")
                nc.sync.dma_start(pt[:], xpr_ap[:, 2 * c0 * P:2 * (c0 * P + cw)])
                tiles.append(t)
                ptiles.append(pt)
                cb.append(c0)
                c0 += csub
                if q == 0:
                    load_consts()
            xpr_t[w] = []
            xf0_t[w] = []
            xf1_t[w] = []
            oh_t[w] = []
            for s in range(WIN_SUB):
                q = 0 if s < CHUNKS0[0] else (1 if s < CHUNKS0[0] + CHUNKS0[1] else 2)
                t = tiles[q]
                cw = CHUNKS0[q] * P
                col = (s - cb[q]) * P
                xpr_t[w].append((ptiles[q], col))
                xf0_t[w].append((t, 0 * cw + col))
                xf1_t[w].append((t, 1 * cw + col))
                oh_t[w].append((t, 2 * cw + col))
        else:
            t = xpool.tile([P, 3 * WIN_NODES], BF16, tag="xw")
            nc.sync.dma_start(
                t[:], xall_ap[:, 3 * w * WIN_NODES:3 * (w + 1) * WIN_NODES])
            pt = xpool.tile([P, 2, WIN_NODES], FP8, tag="xp")
            nc.sync.dma_start(
                pt[:], xpr_ap[:, 2 * w * WIN_NODES:2 * (w + 1) * WIN_NODES])
            xpr_t[w] = [(pt, s * P) for s in range(WIN_SUB)]
            xf0_t[w] = [(t, 0 * WIN_NODES + s * P) for s in range(WIN_SUB)]
            xf1_t[w] = [(t, 1 * WIN_NODES + s * P) for s in range(WIN_SUB)]
            oh_t[w] = [(t, 2 * WIN_NODES + s * P) for s in range(WIN_SUB)]

    seg_tiles = [None] * n_windows
    g_tiles = {}

    def emit_body(w, s):
        ps = ps_sub.tile([P, 2 * HID], F32, tag="ps")
        xp, cp = xpr_t[w][s]
        x0, c0 = xf0_t[w][s]
        x1, c1 = xf1_t[w][s]
        nc.tensor.matmul(ps[:, 0:2 * HID], lhsT=x1[:, c1:c1 + P],
                         rhs=wcat1, start=True, stop=False)
        nc.tensor.matmul(ps[:, 0:HID], lhsT=xp[:, :, cp:cp + P],
                         rhs=w8[:, :, :], start=False, stop=False,
                         perf_mode=mybir.MatmulPerfMode.DoubleRow)
        nc.tensor.matmul(ps[:, HID:2 * HID], lhsT=x0[:, c0:c0 + P],
                         rhs=wlast0, start=False, stop=False)
        nc.tensor.matmul(ps[:, HID:2 * HID], lhsT=ones_t,
                         rhs=bval, start=False, stop=True)
        att = apool.tile([P, HID], BF16, tag="att")
        nc.scalar.activation(att[:], ps[:, 0:HID],
                             mybir.ActivationFunctionType.Sigmoid)
        g_sb = gpool.tile([P, HID], BF16, tag="g")
        nc.vector.tensor_tensor(g_sb[:], att[:], ps[:, HID:2 * HID],
                                op=mybir.AluOpType.mult)
        g_tiles[(w, s)] = g_sb

    def emit_seg(w, s):
        if s == 0:
            seg_tiles[w] = ps_seg.tile([P, HID], F32, tag="seg", name="seg")
        seg = seg_tiles[w]
        oh, co = oh_t[w][s]
        g_sb = g_tiles.pop((w, s))
        nc.tensor.matmul(seg[:, :], lhsT=oh[:, co:co + P], rhs=g_sb[:],
                         start=(s == 0), stop=(s == WIN_SUB - 1))
        if s == WIN_SUB - 1:
            out_t = outpool.tile([P, HID], F32)
            nc.scalar.copy(out_t[:], seg[:, :])
            nc.sync.dma_start(out_ap[w * P:(w + 1) * P, :], out_t[:])

    load_window(0)
    for t in range(n_sub):
        w, s = divmod(t, WIN_SUB)
        if s == 0 and w + 1 < n_windows:
            load_window(w + 1)
        emit_body(w, s)
        if t >= SEGLAG:
            emit_seg(*divmod(t - SEGLAG, WIN_SUB))
    for t in range(n_sub - SEGLAG, n_sub):
        emit_seg(*divmod(t, WIN_SUB))


def build_module(n_windows=WINDOWS_PER_CORE):
    nc = bacc.Bacc("TRN2", debug=False, num_devices=N_CORES)
    nn = n_windows * WIN_NODES
    ins = [
        nc.dram_tensor("xall", [P, 3 * nn], BF16, kind="ExternalInput").ap(),
        nc.dram_tensor("xpr", [P, 2 * nn], FP8, kind="ExternalInput").ap(),
        nc.dram_tensor("wconst", [P, CONST_W], BF16, kind="ExternalInput").ap(),
        nc.dram_tensor("w8", [P, 2 * HID], FP8, kind="ExternalInput").ap(),
    ]
    out_ap = nc.dram_tensor("out", [n_windows * P, HID], F32,
                            kind="ExternalOutput").ap()
    with tile.TileContext(nc) as tc:
        _device_kernel(tc, out_ap, ins, n_windows)
    nc.compile()
    return nc


# ----------------------------------------------------------------------------
# host-side data prep
# ----------------------------------------------------------------------------

def _prep(inputs, n_windows):
    gi = np.asarray(inputs["graph_index"]).astype(np.int64)
    x_in = np.asarray(inputs["input_rep"], dtype=np.float32)
    x_fin = np.asarray(inputs["final_rep"], dtype=np.float32)
    W_lin = np.asarray(inputs["W_lin"], dtype=np.float64)
    b_lin = np.asarray(inputs["b_lin"], dtype=np.float64)
    W_last = np.asarray(inputs["W_last"], dtype=np.float64)
    b_last = np.asarray(inputs["b_last"], dtype=np.float64)

    if np.any(np.diff(gi) < 0):
        order = np.argsort(gi, kind="stable")
        gi = gi[order]
        x_in = x_in[order]
        x_fin = x_fin[order]

    wins = _build_windows(gi, NUM_GRAPHS)
    budget = N_CORES * n_windows
    assert len(wins) <= budget, f"{len(wins)} windows > budget {budget}"
    wins = wins + [(NUM_GRAPHS, 0, len(gi), 0)] * (budget - len(wins))

    # fold b_lin into the node features: min-norm s with W_lin @ s = b_lin
    s_shift = np.linalg.lstsq(W_lin, b_lin, rcond=None)[0]      # [384]
    bval = b_last - s_shift[128:] @ W_last.T                     # [256]
    s32 = s_shift.astype(np.float32)

    xf0_b = (x_fin[:, 0:P] + s32[None, 128:256]).astype(npbf16)
    xf1_b = (x_fin[:, P:2 * P] + s32[None, 256:384]).astype(npbf16)

    WlinT64 = W_lin.T                 # [384, 256] float64
    WlinT = WlinT64.astype(npbf16)
    WlastT = W_last.T.astype(npbf16)  # [256, 256]
    wconst = np.zeros((P, CONST_W), npbf16)
    wconst[:, 0:256] = WlinT[2 * P:3 * P]             # wcat1 att half
    wconst[:, 256:512] = WlastT[P:2 * P]              # wcat1 val half
    wconst[:, 512:768] = WlastT[0:P]                  # wlast0
    wconst[:, 768:896] = np.ones((P, P), npbf16)      # ones
    wconst[:, 896:1152] = np.tile((bval / P)[None, :], (P, 1)).astype(npbf16)

    def f8c(a):
        return np.clip(a, -240.0, 240.0).astype(npf8)

    w8 = np.zeros((P, 2 * HID), npf8)
    w8[:, 0:HID] = f8c(WlinT64[0:P])                  # Wlin0 (xin chunk)
    w8[:, HID:2 * HID] = f8c(WlinT64[P:2 * P])        # Wlin1 (xf0 chunk)

    xin_8 = f8c(x_in + s32[None, :128])
    xf0_8 = f8c(x_fin[:, 0:P] + s32[None, 128:256])
    jgrid = np.arange(P, dtype=np.int32)

    # per-window packed stream layout (must mirror load_window):
    #   xall window blocks [xf0|xf1|oh] bf16, xpr blocks [xin|xf0] fp8
    #   (both chunk-major for window 0)
    nn = n_windows * WIN_NODES
    in_maps = []
    for c in range(N_CORES):
        xall = np.zeros((P, 3 * nn), npbf16)
        xpr = np.zeros((P, 2 * nn), npf8)
        for j in range(n_windows):
            gb, gc, ns, ncnt = wins[c * n_windows + j]
            xf0_w = np.zeros((P, WIN_NODES), npbf16)
            xf1_w = np.zeros((P, WIN_NODES), npbf16)
            xi8_w = np.zeros((P, WIN_NODES), npf8)
            x08_w = np.zeros((P, WIN_NODES), npf8)
            if ncnt > 0:
                xf0_w[:, 0:ncnt] = xf0_b[ns:ns + ncnt].T
                xf1_w[:, 0:ncnt] = xf1_b[ns:ns + ncnt].T
                xi8_w[:, 0:ncnt] = xin_8[ns:ns + ncnt].T
                x08_w[:, 0:ncnt] = xf0_8[ns:ns + ncnt].T
            # one-hot: oh[n, s*128 + j] = (gi_local[s*128+n] == j)
            loc = np.full((WIN_NODES,), -1, np.int32)
            if ncnt > 0:
                loc[0:ncnt] = (gi[ns:ns + ncnt] - gb).astype(np.int32)
            a = loc.reshape(WIN_SUB, P)                      # [s, n]
            ohw = (a[:, :, None] == jgrid[None, None, :])    # [s, n, j]
            oh_w = ohw.transpose(1, 0, 2).reshape(P, WIN_NODES).astype(npbf16)
            off3 = 3 * j * WIN_NODES
            off2 = 2 * j * WIN_NODES
            if j == 0:
                c0 = 0
                for csub in CHUNKS0:
                    cw = csub * P
                    cs = slice(c0 * P, c0 * P + cw)
                    o = off3 + 3 * c0 * P
                    xall[:, o:o + cw] = xf0_w[:, cs]
                    xall[:, o + cw:o + 2 * cw] = xf1_w[:, cs]
                    xall[:, o + 2 * cw:o + 3 * cw] = oh_w[:, cs]
                    o2 = off2 + 2 * c0 * P
                    xpr[:, o2:o2 + cw] = xi8_w[:, cs]
                    xpr[:, o2 + cw:o2 + 2 * cw] = x08_w[:, cs]
                    c0 += csub
            else:
                xall[:, off3:off3 + WIN_NODES] = xf0_w
                xall[:, off3 + WIN_NODES:off3 + 2 * WIN_NODES] = xf1_w
                xall[:, off3 + 2 * WIN_NODES:off3 + 3 * WIN_NODES] = oh_w
                xpr[:, off2:off2 + WIN_NODES] = xi8_w
                xpr[:, off2 + WIN_NODES:off2 + 2 * WIN_NODES] = x08_w
        in_maps.append({"xall": xall, "xpr": xpr,
                        "wconst": wconst, "w8": w8})
    return wins, in_maps


def _assemble(wins, results, n_windows):
    out = np.zeros((NUM_GRAPHS, HID), np.float32)
    for c in range(N_CORES):
        res = results[c]["out"]
        for j in range(n_windows):
            gb, gc, _, _ = wins[c * n_windows + j]
            if gc == 0:
                continue
            out[gb:gb + gc] = res[j * P:j * P + gc]
    return out


# ----------------------------------------------------------------------------
# entry point
# ----------------------------------------------------------------------------

_CACHE = {}
LAST_RESULTS = None


def kernel(**inputs) -> np.ndarray:
    global LAST_RESULTS
    gi = np.asarray(inputs["graph_index"]).astype(np.int64)
    n_wins_needed = len(_build_windows(np.sort(gi), NUM_GRAPHS))
    n_windows = max(WINDOWS_PER_CORE, -(-n_wins_needed // N_CORES))
    if n_windows not in _CACHE:
        _CACHE[n_windows] = build_module(n_windows)
    nc = _CACHE[n_windows]
    wins, in_maps = _prep(inputs, n_windows)
    # a previously-wedged core can fail one run with
    # NRT_EXEC_UNIT_UNRECOVERABLE and reset itself; retry once
    try:
        res = bass_utils.run_bass_kernel_spmd(
            nc, in_maps, core_ids=list(range(N_CORES)))
    except Exception:
        res = bass_utils.run_bass_kernel_spmd(
            nc, in_maps, core_ids=list(range(N_CORES)))
    LAST_RESULTS = res
    return _assemble(wins, res.results, n_windows)
